# revision 9
# baseline (speedup 1.0000x reference)
"""Trainium2 Bass kernel for nn_Decoder (mean-pool L=16 + overlap-add step 8).

Math (per (b, c) slice, est = est_source[b, c] of shape [256, 4000]):
  A[g, f]      = (1/16) * sum_{l=0..15} est[16*g + l, f]          g in 0..15
  out[8*s + j] = A[j, s] + A[8+j, s-1]                            s in 0..4000
with A[., -1] = A[., 4000] = 0 at the edges.  Output length 8*4001 = 32008.

Kernel strategy (8 cores, 4 slices each): the group-of-16 partition reduction
is a matmul with a block 1/16 weight matrix W [128, 8].  We make the frame
axis the matmul output partition dim (lhsT = X tile [128 d, 128 s],
rhs = W [128 d, 8 j]) so PSUM tiles come out as [128 s, 8 j], which maps to
the interleaved DRAM output without any transpose.  The overlap-add is folded
in by accumulating two matmuls per tile into the same PSUM region: the "low"
half (d 0..127) at frame offset s and the "high" half (d 128..255) pre-shifted
by +1 frame on the host.

The host packs both halves (zero-padded to 4096 frames, high half shifted +1)
into one tensor so each slice loads with a single 4 MiB DMA; each slice
stores with a single DMA into a 4096-subframe padded output row (the host
trims to 4001).  Keeping DMA/op counts minimal is also required for
correctness here: walrus allows only one sync-wait on fused fp32 matmuls and
few on DMAs/drain, so the program is structured so no instruction ever needs
more than one.
"""

import sys

if "/opt/trn_rl_repo" not in sys.path:
    sys.path.insert(0, "/opt/trn_rl_repo")

import numpy as np


def _install_ntff_hook():
    """Provide antenv.axon_hooks (absent in this image) so trace=True works.

    The boot-side installer (trn_agent_boot.trn_boot) skips hook setup when
    antenv.axon_hooks is missing; bass_utils then refuses to trace.  We
    register a lazy equivalent backed by the same ctypes NTFF driver.
    """
    import types
    try:
        import antenv
    except ImportError:
        return
    if "antenv.axon_hooks" in sys.modules:
        return
    mod = types.ModuleType("antenv.axon_hooks")
    _state = {}

    def set_axon_ntff_profile_hook(h):
        _state["h"] = h

    def get_axon_ntff_profile_hook():
        if "h" not in _state:
            try:
                from trn_agent_boot.trn_boot import _ntff_profile_via_ctypes
                _state["h"] = _ntff_profile_via_ctypes("/opt/axon/libaxon_pjrt.so")
            except Exception:
                _state["h"] = None
        return _state["h"]

    mod.set_axon_ntff_profile_hook = set_axon_ntff_profile_hook
    mod.get_axon_ntff_profile_hook = get_axon_ntff_profile_hook
    sys.modules["antenv.axon_hooks"] = mod
    antenv.axon_hooks = mod


_install_ntff_hook()

import concourse.bass as bass
import concourse.mybir as mybir
from concourse import tile
from concourse.bass_utils import run_bass_kernel_spmd


class _SingleWaitTileContext(tile.TileContext):
    """TileContext whose kernel-tail drain never carries multiple sem waits.

    The pinned walrus build rejects any instruction with more than one sync
    wait ("Too many sync wait commands").  Tile's default exit emits a single
    Drain waiting on every outstanding proc semaphore.  Instead, emit one
    wait_ge per proc on the SP sequencer (each a single-wait instruction),
    then a wait-free drain.
    """

    def _drain_and_barrier(self, tick_clock, wait_clock):
        nc = self.nc
        clock = tick_clock.global_clock
        sems = wait_clock.sems
        allocated = sems.allocated()
        items = clock.items() if hasattr(clock, "items") else enumerate(clock)
        for proc_idx, tick in items:
            if proc_idx in allocated and tick > 0:
                nc.sync.wait_ge(allocated[proc_idx], tick)
        nc.sync.drain()
        nc.all_engine_barrier()
        popped = nc._tile_sem_poison_stack.pop()
        assert popped is self._sem_poison
        nc.clear_and_free_semaphores(list(self.sems.allocated().values()))
        nc.all_engine_barrier()

# Problem constants (hardcoded per spec)
B, C, D2, FRAMES = 16, 2, 256, 4000
L = 16
SUB = FRAMES + 1          # 4001 output subframes per slice
OUT_LEN = 8 * SUB         # 32008
N_CORES = 8
SLICES = (B * C) // N_CORES   # 4 slices per core
FTILE = 128               # subframes per matmul tile

_CACHE = {}


def _ntiles(frames: int) -> int:
    return -(-(frames + 1) // FTILE)


def _build_w() -> np.ndarray:
    w = np.zeros((128, 8), dtype=np.float32)
    for j in range(8):
        w[16 * j : 16 * j + 16, j] = 1.0 / L
    return w


def _build_nc(frames: int = FRAMES, slices: int = SLICES) -> bass.Bass:
    ntiles = _ntiles(frames)
    padf = ntiles * FTILE

    nc = bass.Bass()
    # Host-packed input: xz[i, d, 0:padf]    = low-half rows, zero-padded;
    #                    xz[i, d, padf:2padf] = high-half rows shifted +1.
    xz_d = nc.dram_tensor("xz", [slices, 128, 2 * padf], mybir.dt.float32,
                          kind="ExternalInput")
    w = nc.dram_tensor("w", [128, 8], mybir.dt.float32, kind="ExternalInput")
    # Padded output: 8*padf per slice; host trims to 8*sub.
    y = nc.dram_tensor("y", [slices, 8 * padf], mybir.dt.float32,
                       kind="ExternalOutput")

    with _SingleWaitTileContext(nc) as tc:
        with (
            tc.tile_pool(name="wp", bufs=1) as wp,
            tc.tile_pool(name="xz", bufs=slices) as xzp,
            tc.tile_pool(name="ob", bufs=slices) as obp,
            tc.tile_pool(name="ps", bufs=slices, space="PSUM") as psp,
            tc.tile_pool(name="wu", bufs=1, space="PSUM") as wup,
        ):
            wt = wp.tile([128, 8], mybir.dt.float32)
            nc.sync.dma_start(out=wt[:], in_=w[:])

            # Warmup matmul: absorbs the W-load DMA wait so no real matmul
            # ever carries two sync waits (walrus limit on fused fp32 MM).
            warm = wup.tile([8, 8], mybir.dt.float32)
            nc.tensor.matmul(warm[:], wt[:], wt[:], start=True, stop=True)

            for i in range(slices):
                xz = xzp.tile([128, 2 * padf], mybir.dt.float32)
                nc.sync.dma_start(out=xz[:], in_=xz_d[i])

                ps = psp.tile([128, 8 * ntiles], mybir.dt.float32)
                for t in range(ntiles):
                    nc.tensor.matmul(
                        ps[:, 8 * t : 8 * t + 8],
                        xz[:, FTILE * t : FTILE * t + FTILE],
                        wt[:],
                        start=True, stop=False,
                    )
                    nc.tensor.matmul(
                        ps[:, 8 * t : 8 * t + 8],
                        xz[:, padf + FTILE * t : padf + FTILE * t + FTILE],
                        wt[:],
                        start=False, stop=True,
                    )

                ob = obp.tile([128, 8 * ntiles], mybir.dt.float32)
                nc.vector.tensor_copy(ob[:], ps[:])

                # Store: y[i] flat idx (FTILE*t + p)*8 + j  <-  ob[p, 8t+j].
                # SWDGE (gpsimd) so these land on the DMASW lane pool,
                # disjoint from the input loads' DMAHW lanes.
                nc.gpsimd.dma_start(
                    out=y[i].rearrange("(t p j) -> p t j", p=128, j=8),
                    in_=ob[:].rearrange("p (t j) -> p t j", j=8),
                )
    return nc


def _get_nc():
    if "nc" not in _CACHE:
        _CACHE["nc"] = _build_nc()
    return _CACHE["nc"]


def _prep_inputs(est: np.ndarray, frames: int, slices_total: int):
    """Pack [S, 256, F] into prepadded low|shifted-high halves [S,128,2*padf]."""
    padf = _ntiles(frames) * FTILE
    xz = np.zeros((slices_total, 128, 2 * padf), dtype=np.float32)
    xz[:, :, :frames] = est[:, 0:128, :]
    xz[:, :, padf + 1 : padf + 1 + frames] = est[:, 128:256, :]
    return xz


def kernel(est_source: np.ndarray, _trace: bool = False) -> np.ndarray:
    est = np.ascontiguousarray(np.asarray(est_source), dtype=np.float32)
    assert est.shape == (B, C, D2, FRAMES)
    flat = est.reshape(B * C, D2, FRAMES)
    xz = _prep_inputs(flat, FRAMES, B * C)
    wmat = _build_w()

    nc = _get_nc()
    in_maps = [
        {"xz": xz[SLICES * k : SLICES * (k + 1)], "w": wmat}
        for k in range(N_CORES)
    ]
    res = run_bass_kernel_spmd(nc, in_maps, core_ids=list(range(N_CORES)),
                               trace=_trace)
    _CACHE["last_results"] = res
    outs = [res.results[k]["y"][:, :OUT_LEN] for k in range(N_CORES)]
    return np.concatenate(outs, axis=0).reshape(B, C, OUT_LEN)


# revision 12
# speedup vs baseline: 1.6297x; 1.6297x over previous
"""Trainium2 Bass kernel for nn_Decoder (mean-pool L=16 + overlap-add step 8).

Math (per (b, c) slice, est = est_source[b, c] of shape [256, 4000]):
  A[g, f]      = (1/16) * sum_{l=0..15} est[16*g + l, f]          g in 0..15
  out[8*s + j] = A[j, s] + A[8+j, s-1]                            s in 0..4000
with A[., -1] = A[., 4000] = 0 at the edges.  Output length 8*4001 = 32008.

Kernel strategy (8 cores, 4 slices each): the group-of-16 partition reduction
is a matmul with a block 1/16 weight matrix W [128, 8].  We make the frame
axis the matmul output partition dim (lhsT = X tile [128 d, 128 s],
rhs = W [128 d, 8 j]) so PSUM tiles come out as [128 s, 8 j], which maps to
the interleaved DRAM output without any transpose.  The overlap-add is folded
in by accumulating two matmuls per tile into the same PSUM region: the "low"
half (d 0..127) at frame offset s and the "high" half (d 128..255) pre-shifted
by +1 frame on the host.

The host packs both halves (zero-padded to 4096 frames, high half shifted +1)
into one tensor so each slice loads with a single 4 MiB DMA; each slice
stores with a single DMA into a 4096-subframe padded output row (the host
trims to 4001).  Keeping DMA/op counts minimal is also required for
correctness here: walrus allows only one sync-wait on fused fp32 matmuls and
few on DMAs/drain, so the program is structured so no instruction ever needs
more than one.
"""

import sys

if "/opt/trn_rl_repo" not in sys.path:
    sys.path.insert(0, "/opt/trn_rl_repo")

import numpy as np


def _install_ntff_hook():
    """Provide antenv.axon_hooks (absent in this image) so trace=True works.

    The boot-side installer (trn_agent_boot.trn_boot) skips hook setup when
    antenv.axon_hooks is missing; bass_utils then refuses to trace.  We
    register a lazy equivalent backed by the same ctypes NTFF driver.
    """
    import types
    try:
        import antenv
    except ImportError:
        return
    if "antenv.axon_hooks" in sys.modules:
        return
    mod = types.ModuleType("antenv.axon_hooks")
    _state = {}

    def set_axon_ntff_profile_hook(h):
        _state["h"] = h

    def get_axon_ntff_profile_hook():
        if "h" not in _state:
            try:
                from trn_agent_boot.trn_boot import _ntff_profile_via_ctypes
                _state["h"] = _ntff_profile_via_ctypes("/opt/axon/libaxon_pjrt.so")
            except Exception:
                _state["h"] = None
        return _state["h"]

    mod.set_axon_ntff_profile_hook = set_axon_ntff_profile_hook
    mod.get_axon_ntff_profile_hook = get_axon_ntff_profile_hook
    sys.modules["antenv.axon_hooks"] = mod
    antenv.axon_hooks = mod


_install_ntff_hook()

import concourse.bass as bass
import concourse.mybir as mybir
from concourse import tile
from concourse.bass_utils import run_bass_kernel_spmd


class _SingleWaitTileContext(tile.TileContext):
    """TileContext whose kernel-tail drain never carries multiple sem waits.

    The pinned walrus build rejects any instruction with more than one sync
    wait ("Too many sync wait commands").  Tile's default exit emits a single
    Drain waiting on every outstanding proc semaphore.  Instead, emit one
    wait_ge per proc on the SP sequencer (each a single-wait instruction),
    then a wait-free drain.
    """

    # proc indices >= _FIRST_DMA_PROC are DMA lanes whose semaphores advance
    # by 16 per op (one inc per SDMA engine) while the vector clock ticks 1.
    _FIRST_DMA_PROC = 11

    def _drain_and_barrier(self, tick_clock, wait_clock):
        nc = self.nc
        clock = tick_clock.global_clock  # bass_rust.VectorClock: 27 ints
        allocated = wait_clock.sems.allocated()
        for proc_idx, tick in enumerate(clock):
            if tick > 0 and proc_idx in allocated:
                val = tick * 16 if proc_idx >= self._FIRST_DMA_PROC else tick
                nc.sync.wait_ge(allocated[proc_idx], val)
        nc.sync.drain()
        nc.all_engine_barrier()
        popped = nc._tile_sem_poison_stack.pop()
        assert popped is self._sem_poison
        nc.clear_and_free_semaphores(list(self.sems.allocated().values()))
        nc.all_engine_barrier()

# Problem constants (hardcoded per spec)
B, C, D2, FRAMES = 16, 2, 256, 4000
L = 16
SUB = FRAMES + 1          # 4001 output subframes per slice
OUT_LEN = 8 * SUB         # 32008
N_CORES = 8
SLICES = (B * C) // N_CORES   # 4 slices per core
FTILE = 128               # subframes per matmul tile

_CACHE = {}


def _ntiles(frames: int) -> int:
    return -(-(frames + 1) // FTILE)


def _build_w() -> np.ndarray:
    w = np.zeros((128, 8), dtype=np.float32)
    for j in range(8):
        w[16 * j : 16 * j + 16, j] = 1.0 / L
    return w


def _build_nc(frames: int = FRAMES, slices: int = SLICES,
              mm_dt=None) -> bass.Bass:
    # mm_dt: matmul operand dtype; float32 is exact but the PE lowers it to
    # two half-speed passes.  float32r (same bits, tf32-like multiply,
    # ~1e-4 rel err) runs the PE twice as fast.
    if mm_dt is None:
        mm_dt = mybir.dt.float32r
    ntiles = _ntiles(frames)
    padf = ntiles * FTILE

    nc = bass.Bass()
    # Host-packed input: xz[i, d, 0:padf]    = low-half rows, zero-padded;
    #                    xz[i, d, padf:2padf] = high-half rows shifted +1.
    xz_d = nc.dram_tensor("xz", [slices, 128, 2 * padf], mm_dt,
                          kind="ExternalInput")
    w = nc.dram_tensor("w", [128, 8], mm_dt, kind="ExternalInput")
    # Padded output: 8*padf per slice; host trims to 8*sub.
    y = nc.dram_tensor("y", [slices, 8 * padf], mybir.dt.float32,
                       kind="ExternalOutput")

    with _SingleWaitTileContext(nc) as tc:
        with (
            tc.tile_pool(name="wp", bufs=1) as wp,
            tc.tile_pool(name="xz", bufs=slices) as xzp,
            tc.tile_pool(name="ob", bufs=slices) as obp,
            tc.tile_pool(name="ps", bufs=slices, space="PSUM") as psp,
            tc.tile_pool(name="wu", bufs=1, space="PSUM") as wup,
        ):
            wt = wp.tile([128, 8], mm_dt)
            nc.sync.dma_start(out=wt[:], in_=w[:])

            # Warmup matmul: absorbs the W-load DMA wait so no real matmul
            # ever carries two sync waits (walrus limit on fused fp32 MM).
            warm = wup.tile([8, 8], mybir.dt.float32)
            nc.tensor.matmul(warm[:], wt[:], wt[:], start=True, stop=True)

            for i in range(slices):
                xz = xzp.tile([128, 2 * padf], mm_dt)
                nc.sync.dma_start(out=xz[:], in_=xz_d[i])

                ps = psp.tile([128, 8 * ntiles], mybir.dt.float32)
                for t in range(ntiles):
                    nc.tensor.matmul(
                        ps[:, 8 * t : 8 * t + 8],
                        xz[:, FTILE * t : FTILE * t + FTILE],
                        wt[:],
                        start=True, stop=False,
                    )
                    nc.tensor.matmul(
                        ps[:, 8 * t : 8 * t + 8],
                        xz[:, padf + FTILE * t : padf + FTILE * t + FTILE],
                        wt[:],
                        start=False, stop=True,
                    )

                ob = obp.tile([128, 8 * ntiles], mybir.dt.float32)
                nc.vector.tensor_copy(ob[:], ps[:])

                # Store: y[i] flat idx (FTILE*t + p)*8 + j  <-  ob[p, 8t+j].
                # SWDGE (gpsimd) so these land on the DMASW lane pool,
                # disjoint from the input loads' DMAHW lanes.
                nc.gpsimd.dma_start(
                    out=y[i].rearrange("(t p j) -> p t j", p=128, j=8),
                    in_=ob[:].rearrange("p (t j) -> p t j", j=8),
                )
    return nc


def _get_nc():
    if "nc" not in _CACHE:
        _CACHE["nc"] = _build_nc()
    return _CACHE["nc"]


def _prep_inputs(est: np.ndarray, frames: int, slices_total: int):
    """Pack [S, 256, F] into prepadded low|shifted-high halves [S,128,2*padf]."""
    padf = _ntiles(frames) * FTILE
    xz = np.zeros((slices_total, 128, 2 * padf), dtype=np.float32)
    xz[:, :, :frames] = est[:, 0:128, :]
    xz[:, :, padf + 1 : padf + 1 + frames] = est[:, 128:256, :]
    return xz


def kernel(est_source: np.ndarray, _trace: bool = False) -> np.ndarray:
    est = np.ascontiguousarray(np.asarray(est_source), dtype=np.float32)
    assert est.shape == (B, C, D2, FRAMES)
    flat = est.reshape(B * C, D2, FRAMES)
    xz = _prep_inputs(flat, FRAMES, B * C)
    wmat = _build_w()

    nc = _get_nc()
    in_maps = [
        {"xz": xz[SLICES * k : SLICES * (k + 1)], "w": wmat}
        for k in range(N_CORES)
    ]
    res = run_bass_kernel_spmd(nc, in_maps, core_ids=list(range(N_CORES)),
                               trace=_trace)
    _CACHE["last_results"] = res
    outs = [res.results[k]["y"][:, :OUT_LEN] for k in range(N_CORES)]
    return np.concatenate(outs, axis=0).reshape(B, C, OUT_LEN)
